# revision 3
# baseline (speedup 1.0000x reference)
"""Multi-head attention (B=4, S=2048, D=1024, H=16, Hd=64) on 8 trn2 cores.

Sharding: core c = (batch b = c // 2, head-group hg = c % 2). Each core
computes attention for 8 heads of one batch and the corresponding slice of
the output projection; host sums the two partial outputs per batch.

Per-core layout (all matmuls bf16 with fp32 PSUM accumulation):
  xt   = x[b].T                    [D=1024, S=2048]  (lhsT/rhs K-major)
  qT/kT = (Wslice.T @ .. )         [512, 2048]  d-major, 4 pair-tiles of 128
  v    = x @ Wv_slice              [2048, 512]  s-major
  per head: scoresT[k,q] tile = kT.T-block @ qT  -> exp (ScalarE, scale=1/8)
            outT[d,q] += v-block.T @ expT ; denom[q] += ones.T @ expT
  normalize: outT *= broadcast(1/denom) via K=1 ones-matmul
  y = outT.T-blocks @ Wo_slice + bo   [2048, 1024] fp32 partial
"""

import numpy as np
import ml_dtypes

S = 2048
D = 1024
HG_D = 512          # head dims per core (8 heads x 64)
NH = 8              # heads per core
KT = S // 128       # 16 k-tiles
DT = D // 128       # 8 contraction tiles for QKV
ST = S // 128       # 16 s-tiles
OT = HG_D // 128    # 4 contraction tiles for O-proj / pair tiles
N_CORES = 8

BF16 = ml_dtypes.bfloat16

_CACHED_NC = None


def _build_nc():
    import concourse.bass as bass  # noqa: F401
    import concourse.mybir as mybir
    import concourse.tile as tile
    from concourse import bacc

    f32 = mybir.dt.float32
    bf16 = mybir.dt.bfloat16
    Exp = mybir.ActivationFunctionType.Exp

    nc = bacc.Bacc("TRN2", target_bir_lowering=False, debug=False,
                   num_devices=N_CORES)

    xt_d = nc.dram_tensor("xt", [D, S], bf16, kind="ExternalInput")
    wq_d = nc.dram_tensor("wq", [D, HG_D], bf16, kind="ExternalInput")
    wk_d = nc.dram_tensor("wk", [D, HG_D], bf16, kind="ExternalInput")
    wv_d = nc.dram_tensor("wv", [D, HG_D], bf16, kind="ExternalInput")
    wo_d = nc.dram_tensor("wo", [HG_D, D], bf16, kind="ExternalInput")
    bq_d = nc.dram_tensor("bqt", [128, OT], f32, kind="ExternalInput")
    bk_d = nc.dram_tensor("bkt", [128, OT], f32, kind="ExternalInput")
    bv_d = nc.dram_tensor("bvr", [1, HG_D], bf16, kind="ExternalInput")
    bo_d = nc.dram_tensor("bor", [1, D], bf16, kind="ExternalInput")
    y_d = nc.dram_tensor("y", [S, D], f32, kind="ExternalOutput")

    with tile.TileContext(nc) as tc:
        with (
            tc.tile_pool(name="cpool", bufs=1) as cpool,
            tc.tile_pool(name="wpool", bufs=2) as wpool,
            tc.tile_pool(name="pspool", bufs=2, space="PSUM") as pspool,
            tc.tile_pool(name="popool", bufs=1, space="PSUM") as popool,
            tc.tile_pool(name="pdpool", bufs=1, space="PSUM") as pdpool,
        ):
            # ---- persistent SBUF tiles ----
            xt_sb = cpool.tile([128, DT, S], bf16, name="xt_sb")
            wq_sb = cpool.tile([128, DT, HG_D], bf16, name="wq_sb")
            wk_sb = cpool.tile([128, DT, HG_D], bf16, name="wk_sb")
            wv_sb = cpool.tile([128, DT, HG_D], bf16, name="wv_sb")
            wo_sb = cpool.tile([128, OT, D], bf16, name="wo_sb")
            bq_sb = cpool.tile([128, OT], f32, name="bq_sb")
            bk_sb = cpool.tile([128, OT], f32, name="bk_sb")
            bvr_sb = cpool.tile([1, HG_D], bf16, name="bvr_sb")
            bor_sb = cpool.tile([1, D], bf16, name="bor_sb")
            ones_t = cpool.tile([128, 128], bf16, name="ones_t")
            qT_sb = cpool.tile([128, OT, S], bf16, name="qT_sb")
            kT_sb = cpool.tile([128, OT, S], bf16, name="kT_sb")
            v_sb = cpool.tile([128, ST, HG_D], bf16, name="v_sb")
            aoT_sb = cpool.tile([128, OT, S], bf16, name="aoT_sb")

            # ---- loads ----
            for k in range(DT):
                nc.sync.dma_start(out=xt_sb[:, k, :], in_=xt_d[k * 128:(k + 1) * 128, :])
                nc.sync.dma_start(out=wq_sb[:, k, :], in_=wq_d[k * 128:(k + 1) * 128, :])
                nc.sync.dma_start(out=wk_sb[:, k, :], in_=wk_d[k * 128:(k + 1) * 128, :])
                nc.sync.dma_start(out=wv_sb[:, k, :], in_=wv_d[k * 128:(k + 1) * 128, :])
            for k in range(OT):
                nc.sync.dma_start(out=wo_sb[:, k, :], in_=wo_d[k * 128:(k + 1) * 128, :])
            nc.sync.dma_start(out=bq_sb[:], in_=bq_d[:])
            nc.sync.dma_start(out=bk_sb[:], in_=bk_d[:])
            nc.sync.dma_start(out=bvr_sb[:], in_=bv_d[:])
            nc.sync.dma_start(out=bor_sb[:], in_=bo_d[:])
            nc.gpsimd.memset(ones_t[:], 1.0)

            # ---- projections: qT/kT [d-major], bias via per-partition scalar ----
            for w_sb, b_sb, out_sb in ((wq_sb, bq_sb, qT_sb), (wk_sb, bk_sb, kT_sb)):
                for p in range(OT):
                    for jc in range(4):
                        pq = pspool.tile([128, 512], f32, tag="ps", name="pq")
                        for k in range(DT):
                            nc.tensor.matmul(
                                pq[:],
                                w_sb[:, k, p * 128:(p + 1) * 128],
                                xt_sb[:, k, jc * 512:(jc + 1) * 512],
                                start=(k == 0), stop=(k == DT - 1),
                            )
                        nc.vector.tensor_scalar_add(
                            out_sb[:, p, jc * 512:(jc + 1) * 512], pq[:], b_sb[:, p:p + 1])

            # ---- v projection [s-major], bias via K=1 ones matmul ----
            for st in range(ST):
                pv = pspool.tile([128, 512], f32, tag="ps", name="pv")
                for k in range(DT):
                    nc.tensor.matmul(
                        pv[:],
                        xt_sb[:, k, st * 128:(st + 1) * 128],
                        wv_sb[:, k, :],
                        start=(k == 0), stop=False,
                    )
                nc.tensor.matmul(pv[:], ones_t[0:1, 0:128], bvr_sb[0:1, :],
                                 start=False, stop=True)
                nc.vector.tensor_copy(v_sb[:, st, :], pv[:])

            # ---- attention, one head pair at a time ----
            for p in range(OT):
                rf = wpool.tile([65, S], f32, tag="rf", name="rf")
                rb = wpool.tile([65, S], bf16, tag="rb", name="rb")
                for j in range(2):
                    ot = popool.tile([128, 1024], f32, tag="po", name="ot")
                    for hh in range(2):
                        h = 2 * p + hh
                        off = 64 * hh
                        dr = 64 - 32 * hh
                        dt_ = pdpool.tile([65, 1024], f32, tag="pd", name="dt")
                        for i in range(KT):
                            stt = pspool.tile([128, 1024], f32, tag="ps", name="stt")
                            for l in range(2):
                                nc.tensor.matmul(
                                    stt[:, l * 512:(l + 1) * 512],
                                    kT_sb[off:off + 64, p, i * 128:(i + 1) * 128],
                                    qT_sb[off:off + 64, p,
                                          j * 1024 + l * 512:j * 1024 + (l + 1) * 512],
                                    start=True, stop=True,
                                )
                            et = wpool.tile([128, 1024], bf16, tag="exp", bufs=3,
                                            name="et")
                            nc.scalar.activation(et[:], stt[:], Exp, scale=0.125)
                            for l in range(2):
                                nc.tensor.matmul(
                                    ot[off:off + 64, l * 512:(l + 1) * 512],
                                    v_sb[:, i, h * 64:(h + 1) * 64],
                                    et[:, l * 512:(l + 1) * 512],
                                    start=(i == 0), stop=(i == KT - 1),
                                )
                                nc.tensor.matmul(
                                    dt_[dr:dr + 1, l * 512:(l + 1) * 512],
                                    ones_t[:, 0:1],
                                    et[:, l * 512:(l + 1) * 512],
                                    start=(i == 0), stop=(i == KT - 1),
                                )
                        nc.vector.reciprocal(
                            rf[dr:dr + 1, j * 1024:(j + 1) * 1024],
                            dt_[dr:dr + 1, :])
                        nc.vector.tensor_copy(
                            rb[dr:dr + 1, j * 1024:(j + 1) * 1024],
                            rf[dr:dr + 1, j * 1024:(j + 1) * 1024])
                    # pair done for this q-chunk: drain, broadcast 1/denom, scale
                    nc.vector.tensor_copy(
                        aoT_sb[:, p, j * 1024:(j + 1) * 1024], ot[:])
                    bt = pspool.tile([128, 1024], f32, tag="ps", name="bt")
                    for off2, dr2 in ((0, 64), (64, 32)):
                        for l in range(2):
                            nc.tensor.matmul(
                                bt[off2:off2 + 64, l * 512:(l + 1) * 512],
                                ones_t[dr2:dr2 + 1, 0:64],
                                rb[dr2:dr2 + 1,
                                   j * 1024 + l * 512:j * 1024 + (l + 1) * 512],
                                start=True, stop=True,
                            )
                    nc.vector.tensor_mul(
                        aoT_sb[:, p, j * 1024:(j + 1) * 1024],
                        aoT_sb[:, p, j * 1024:(j + 1) * 1024],
                        bt[:])

            # ---- output projection + bias ----
            for st in range(ST):
                yt = wpool.tile([128, D], f32, tag="y", name="yt")
                for l in range(2):
                    py = pspool.tile([128, 512], f32, tag="ps", name="py")
                    for kt in range(OT):
                        nc.tensor.matmul(
                            py[:],
                            aoT_sb[:, kt, st * 128:(st + 1) * 128],
                            wo_sb[:, kt, l * 512:(l + 1) * 512],
                            start=(kt == 0), stop=False,
                        )
                    nc.tensor.matmul(py[:], ones_t[0:1, 0:128],
                                     bor_sb[0:1, l * 512:(l + 1) * 512],
                                     start=False, stop=True)
                    nc.vector.tensor_copy(yt[:, l * 512:(l + 1) * 512], py[:])
                nc.sync.dma_start(out=y_d[st * 128:(st + 1) * 128, :], in_=yt[:])

    nc.compile()
    return nc


def get_nc():
    global _CACHED_NC
    if _CACHED_NC is None:
        _CACHED_NC = _build_nc()
    return _CACHED_NC


def make_in_maps(x, Wq, bq, Wk, bk, Wv, bv, Wo, bo):
    x = np.asarray(x, dtype=np.float32)
    in_maps = []
    for c in range(N_CORES):
        b, hg = c // 2, c % 2
        sl = slice(hg * HG_D, (hg + 1) * HG_D)
        in_maps.append({
            "xt": np.ascontiguousarray(np.asarray(x[b]).T).astype(BF16),
            "wq": np.ascontiguousarray(np.asarray(Wq)[:, sl]).astype(BF16),
            "wk": np.ascontiguousarray(np.asarray(Wk)[:, sl]).astype(BF16),
            "wv": np.ascontiguousarray(np.asarray(Wv)[:, sl]).astype(BF16),
            "wo": np.ascontiguousarray(np.asarray(Wo)[sl, :]).astype(BF16),
            "bqt": np.ascontiguousarray(
                np.asarray(bq, np.float32)[sl].reshape(OT, 128).T),
            "bkt": np.ascontiguousarray(
                np.asarray(bk, np.float32)[sl].reshape(OT, 128).T),
            "bvr": np.asarray(bv, np.float32)[sl].reshape(1, HG_D).astype(BF16),
            "bor": (np.asarray(bo, np.float32) if hg == 0
                    else np.zeros(D, np.float32)).reshape(1, D).astype(BF16),
        })
    return in_maps


def run_cores(in_maps, trace=False):
    try:
        import ntff_shim
        ntff_shim.install()
    except Exception:
        pass
    from concourse.bass_utils import run_bass_kernel_spmd

    nc = get_nc()
    return run_bass_kernel_spmd(nc, in_maps, list(range(N_CORES)), trace=trace)


def combine(results):
    y = np.empty((4, S, D), np.float32)
    for b in range(4):
        y[b] = results[2 * b]["y"] + results[2 * b + 1]["y"]
    return y


def kernel(x, Wq, bq, Wk, bk, Wv, bv, Wo, bo):
    in_maps = make_in_maps(x, Wq, bq, Wk, bk, Wv, bv, Wo, bo)
    res = run_cores(in_maps, trace=False)
    return combine(res.results)


# revision 7
# speedup vs baseline: 1.3215x; 1.3215x over previous
"""Multi-head attention (B=4, S=2048, D=1024, H=16, Hd=64) on 8 trn2 cores.

Sharding: core c = (batch b = c // 2, head-group hg = c % 2). Each core
computes attention for 8 heads of one batch and the corresponding slice of
the output projection; host sums the two partial outputs per batch.

Per-core layout (all matmuls bf16 with fp32 PSUM accumulation):
  xt   = x[b].T                    [D=1024, S=2048]  (lhsT/rhs K-major)
  qT/kT = (Wslice.T @ .. )         [512, 2048]  d-major, 4 pair-tiles of 128
  v    = x @ Wv_slice              [2048, 512]  s-major
  per head: scoresT[k,q] tile = kT.T-block @ qT  -> exp (ScalarE, scale=1/8)
            outT[d,q] += v-block.T @ expT ; denom[q] += ones.T @ expT
  normalize: outT *= broadcast(1/denom) via K=1 ones-matmul
  y = outT.T-blocks @ Wo_slice + bo   [2048, 1024] fp32 partial
"""

import numpy as np
import ml_dtypes

S = 2048
D = 1024
HG_D = 512          # head dims per core (8 heads x 64)
NH = 8              # heads per core
KT = S // 128       # 16 k-tiles
DT = D // 128       # 8 contraction tiles for QKV
ST = S // 128       # 16 s-tiles
OT = HG_D // 128    # 4 contraction tiles for O-proj / pair tiles
N_CORES = 8

BF16 = ml_dtypes.bfloat16

_CACHED_NC = None


def _build_nc():
    import concourse.bass as bass  # noqa: F401
    import concourse.mybir as mybir
    import concourse.tile as tile
    from concourse import bacc

    f32 = mybir.dt.float32
    bf16 = mybir.dt.bfloat16
    Exp = mybir.ActivationFunctionType.Exp

    nc = bacc.Bacc("TRN2", target_bir_lowering=False, debug=False,
                   num_devices=N_CORES)

    xt_d = nc.dram_tensor("xt", [D, S], bf16, kind="ExternalInput")
    wq_d = nc.dram_tensor("wq", [D, HG_D], bf16, kind="ExternalInput")
    wk_d = nc.dram_tensor("wk", [D, HG_D], bf16, kind="ExternalInput")
    wv_d = nc.dram_tensor("wv", [D, HG_D], bf16, kind="ExternalInput")
    wo_d = nc.dram_tensor("wo", [HG_D, D], bf16, kind="ExternalInput")
    bq_d = nc.dram_tensor("bqt", [128, OT], f32, kind="ExternalInput")
    bk_d = nc.dram_tensor("bkt", [128, OT], f32, kind="ExternalInput")
    bv_d = nc.dram_tensor("bvr", [1, HG_D], bf16, kind="ExternalInput")
    bo_d = nc.dram_tensor("bor", [1, D], bf16, kind="ExternalInput")
    y_d = nc.dram_tensor("y", [S, D], f32, kind="ExternalOutput")

    with tile.TileContext(nc) as tc:
        with (
            tc.tile_pool(name="cpool", bufs=1) as cpool,
            tc.tile_pool(name="wpool", bufs=2) as wpool,
            tc.tile_pool(name="pspool", bufs=2, space="PSUM") as pspool,
            tc.tile_pool(name="popool", bufs=2, space="PSUM") as popool,
        ):
            # ---- persistent SBUF tiles ----
            xt_sb = cpool.tile([128, DT, S], bf16, name="xt_sb")
            wq_sb = cpool.tile([128, DT, HG_D], bf16, name="wq_sb")
            wk_sb = cpool.tile([128, DT, HG_D], bf16, name="wk_sb")
            wv_sb = cpool.tile([128, DT, HG_D], bf16, name="wv_sb")
            wo_sb = cpool.tile([128, OT, D], bf16, name="wo_sb")
            bq_sb = cpool.tile([128, OT], f32, name="bq_sb")
            bk_sb = cpool.tile([128, OT], f32, name="bk_sb")
            bvr_sb = cpool.tile([1, HG_D], bf16, name="bvr_sb")
            bor_sb = cpool.tile([1, D], bf16, name="bor_sb")
            ones_t = cpool.tile([128, 128], bf16, name="ones_t")
            qT_sb = cpool.tile([128, OT, S], bf16, name="qT_sb")
            kT_sb = cpool.tile([128, OT, S], bf16, name="kT_sb")
            # v with a trailing ones column per head: attnv lhsT [128, 65]
            # whose 65th output row accumulates the softmax denominator.
            v_sb = cpool.tile([128, ST, NH, 65], bf16, name="v_sb")
            aoT_sb = cpool.tile([128, OT, S], bf16, name="aoT_sb")

            # ---- loads ----
            for k in range(DT):
                nc.sync.dma_start(out=xt_sb[:, k, :], in_=xt_d[k * 128:(k + 1) * 128, :])
                nc.sync.dma_start(out=wq_sb[:, k, :], in_=wq_d[k * 128:(k + 1) * 128, :])
                nc.sync.dma_start(out=wk_sb[:, k, :], in_=wk_d[k * 128:(k + 1) * 128, :])
                nc.sync.dma_start(out=wv_sb[:, k, :], in_=wv_d[k * 128:(k + 1) * 128, :])
            for k in range(OT):
                nc.sync.dma_start(out=wo_sb[:, k, :], in_=wo_d[k * 128:(k + 1) * 128, :])
            nc.sync.dma_start(out=bq_sb[:], in_=bq_d[:])
            nc.sync.dma_start(out=bk_sb[:], in_=bk_d[:])
            nc.sync.dma_start(out=bvr_sb[:], in_=bv_d[:])
            nc.sync.dma_start(out=bor_sb[:], in_=bo_d[:])
            nc.gpsimd.memset(ones_t[:], 1.0)

            # ---- projections: qT/kT [d-major], bias via per-partition scalar ----
            for w_sb, b_sb, out_sb in ((wq_sb, bq_sb, qT_sb), (wk_sb, bk_sb, kT_sb)):
                for p in range(OT):
                    for jc in range(4):
                        pq = pspool.tile([128, 512], f32, tag="ps", name="pq")
                        for k in range(DT):
                            nc.tensor.matmul(
                                pq[:],
                                w_sb[:, k, p * 128:(p + 1) * 128],
                                xt_sb[:, k, jc * 512:(jc + 1) * 512],
                                start=(k == 0), stop=(k == DT - 1),
                            )
                        nc.vector.tensor_scalar_add(
                            out_sb[:, p, jc * 512:(jc + 1) * 512], pq[:], b_sb[:, p:p + 1])

            # ---- v projection [s-major], bias via K=1 ones matmul ----
            nc.vector.memset(v_sb[:], 1.0)
            for st in range(ST):
                pv = pspool.tile([128, 512], f32, tag="ps", name="pv")
                for k in range(DT):
                    nc.tensor.matmul(
                        pv[:],
                        xt_sb[:, k, st * 128:(st + 1) * 128],
                        wv_sb[:, k, :],
                        start=(k == 0), stop=False,
                    )
                nc.tensor.matmul(pv[:], ones_t[0:1, 0:128], bvr_sb[0:1, :],
                                 start=False, stop=True)
                nc.vector.tensor_copy(
                    v_sb[:, st, :, 0:64],
                    pv.rearrange("p (h c) -> p h c", c=64))

            # ---- attention, head by head ----
            for h in range(NH):
                p, hh = h // 2, h % 2
                off = 64 * hh
                for j in range(2):
                    jb = slice(j * 1024, (j + 1) * 1024)
                    ot = popool.tile([65, 1024], f32, tag="po", name="ot")
                    for i in range(KT):
                        stt = pspool.tile([128, 1024], f32, tag="ps", name="stt")
                        for l in range(2):
                            nc.tensor.matmul(
                                stt[:, l * 512:(l + 1) * 512],
                                kT_sb[off:off + 64, p, i * 128:(i + 1) * 128],
                                qT_sb[off:off + 64, p,
                                      j * 1024 + l * 512:j * 1024 + (l + 1) * 512],
                                start=True, stop=True,
                            )
                        et = wpool.tile([128, 1024], bf16, tag="exp", bufs=3,
                                        name="et")
                        nc.scalar.activation(et[:], stt[:], Exp, scale=0.125)
                        for l in range(2):
                            nc.tensor.matmul(
                                ot[:, l * 512:(l + 1) * 512],
                                v_sb[:, i, h, :],
                                et[:, l * 512:(l + 1) * 512],
                                start=(i == 0), stop=(i == KT - 1),
                            )
                    # drain attn rows (cross-partition for odd heads), then
                    # 1/denom from row 64, broadcast it, and scale in place.
                    nc.vector.tensor_copy(aoT_sb[off:off + 64, p, jb], ot[0:64, :])
                    rf = wpool.tile([65, 1024], f32, tag="rf", name="rf")
                    rb = wpool.tile([65, 1024], bf16, tag="rb", name="rb")
                    nc.vector.reciprocal(rf[64:65, :], ot[64:65, :])
                    nc.vector.tensor_copy(rb[64:65, :], rf[64:65, :])
                    bt = pspool.tile([128, 1024], f32, tag="ps", name="bt")
                    for l in range(2):
                        nc.tensor.matmul(
                            bt[off:off + 64, l * 512:(l + 1) * 512],
                            ones_t[64:65, 0:64],
                            rb[64:65, l * 512:(l + 1) * 512],
                            start=True, stop=True,
                        )
                    nc.vector.tensor_mul(
                        aoT_sb[off:off + 64, p, jb],
                        aoT_sb[off:off + 64, p, jb],
                        bt[off:off + 64, :])

            # ---- output projection + bias ----
            for st in range(ST):
                yt = wpool.tile([128, D], f32, tag="y", name="yt")
                for l in range(2):
                    py = pspool.tile([128, 512], f32, tag="ps", name="py")
                    for kt in range(OT):
                        nc.tensor.matmul(
                            py[:],
                            aoT_sb[:, kt, st * 128:(st + 1) * 128],
                            wo_sb[:, kt, l * 512:(l + 1) * 512],
                            start=(kt == 0), stop=False,
                        )
                    nc.tensor.matmul(py[:], ones_t[0:1, 0:128],
                                     bor_sb[0:1, l * 512:(l + 1) * 512],
                                     start=False, stop=True)
                    nc.vector.tensor_copy(yt[:, l * 512:(l + 1) * 512], py[:])
                nc.sync.dma_start(out=y_d[st * 128:(st + 1) * 128, :], in_=yt[:])

    nc.compile()
    return nc


def get_nc():
    global _CACHED_NC
    if _CACHED_NC is None:
        _CACHED_NC = _build_nc()
    return _CACHED_NC


def make_in_maps(x, Wq, bq, Wk, bk, Wv, bv, Wo, bo):
    x = np.asarray(x, dtype=np.float32)
    in_maps = []
    for c in range(N_CORES):
        b, hg = c // 2, c % 2
        sl = slice(hg * HG_D, (hg + 1) * HG_D)
        in_maps.append({
            "xt": np.ascontiguousarray(np.asarray(x[b]).T).astype(BF16),
            "wq": np.ascontiguousarray(np.asarray(Wq)[:, sl]).astype(BF16),
            "wk": np.ascontiguousarray(np.asarray(Wk)[:, sl]).astype(BF16),
            "wv": np.ascontiguousarray(np.asarray(Wv)[:, sl]).astype(BF16),
            "wo": np.ascontiguousarray(np.asarray(Wo)[sl, :]).astype(BF16),
            "bqt": np.ascontiguousarray(
                np.asarray(bq, np.float32)[sl].reshape(OT, 128).T),
            "bkt": np.ascontiguousarray(
                np.asarray(bk, np.float32)[sl].reshape(OT, 128).T),
            "bvr": np.asarray(bv, np.float32)[sl].reshape(1, HG_D).astype(BF16),
            "bor": (np.asarray(bo, np.float32) if hg == 0
                    else np.zeros(D, np.float32)).reshape(1, D).astype(BF16),
        })
    return in_maps


def run_cores(in_maps, trace=False):
    try:
        import ntff_shim
        ntff_shim.install()
    except Exception:
        pass
    from concourse.bass_utils import run_bass_kernel_spmd

    nc = get_nc()
    return run_bass_kernel_spmd(nc, in_maps, list(range(N_CORES)), trace=trace)


def combine(results):
    y = np.empty((4, S, D), np.float32)
    for b in range(4):
        y[b] = results[2 * b]["y"] + results[2 * b + 1]["y"]
    return y


def kernel(x, Wq, bq, Wk, bk, Wv, bv, Wo, bo):
    in_maps = make_in_maps(x, Wq, bq, Wk, bk, Wv, bv, Wo, bo)
    res = run_cores(in_maps, trace=False)
    return combine(res.results)


# revision 8
# speedup vs baseline: 1.8611x; 1.4084x over previous
"""Multi-head attention (B=4, S=2048, D=1024, H=16, Hd=64) on 8 trn2 cores.

Sharding: core c = (batch b = c // 2, head-group hg = c % 2). Each core
computes attention for 8 heads of one batch and the corresponding slice of
the output projection; host sums the two partial outputs per batch.

Per-core layout (all matmuls bf16 with fp32 PSUM accumulation):
  xt   = x[b].T                    [D=1024, S=2048]  (lhsT/rhs K-major)
  qT/kT = (Wslice.T @ .. )         [512, 2048]  d-major, 4 pair-tiles of 128
  v    = x @ Wv_slice              [2048, 512]  s-major
  per head: scoresT[k,q] tile = kT.T-block @ qT  -> exp (ScalarE, scale=1/8)
            outT[d,q] += v-block.T @ expT ; denom[q] += ones.T @ expT
  normalize: outT *= broadcast(1/denom) via K=1 ones-matmul
  y = outT.T-blocks @ Wo_slice + bo   [2048, 1024] fp32 partial
"""

import numpy as np
import ml_dtypes

S = 2048
D = 1024
HG_D = 512          # head dims per core (8 heads x 64)
NH = 8              # heads per core
KT = S // 128       # 16 k-tiles
DT = D // 128       # 8 contraction tiles for QKV
ST = S // 128       # 16 s-tiles
OT = HG_D // 128    # 4 contraction tiles for O-proj / pair tiles
N_CORES = 8

BF16 = ml_dtypes.bfloat16

_CACHED_NC = None


def _build_nc():
    import concourse.bass as bass  # noqa: F401
    import concourse.mybir as mybir
    import concourse.tile as tile
    from concourse import bacc

    f32 = mybir.dt.float32
    bf16 = mybir.dt.bfloat16
    Exp = mybir.ActivationFunctionType.Exp

    nc = bacc.Bacc("TRN2", target_bir_lowering=False, debug=False,
                   num_devices=N_CORES)

    xt_d = nc.dram_tensor("xt", [D, S], bf16, kind="ExternalInput")
    wq_d = nc.dram_tensor("wq", [D, HG_D], bf16, kind="ExternalInput")
    wk_d = nc.dram_tensor("wk", [D, HG_D], bf16, kind="ExternalInput")
    wv_d = nc.dram_tensor("wv", [D, HG_D], bf16, kind="ExternalInput")
    wo_d = nc.dram_tensor("wo", [HG_D, D], bf16, kind="ExternalInput")
    bq_d = nc.dram_tensor("bqt", [128, OT], f32, kind="ExternalInput")
    bk_d = nc.dram_tensor("bkt", [128, OT], f32, kind="ExternalInput")
    bv_d = nc.dram_tensor("bvr", [1, HG_D], bf16, kind="ExternalInput")
    bo_d = nc.dram_tensor("bor", [1, D], bf16, kind="ExternalInput")
    y_d = nc.dram_tensor("y", [S, D], f32, kind="ExternalOutput")

    with tile.TileContext(nc) as tc:
        with (
            tc.tile_pool(name="cpool", bufs=1) as cpool,
            tc.tile_pool(name="wpool", bufs=2) as wpool,
            tc.tile_pool(name="pspool", bufs=2, space="PSUM") as pspool,
            tc.tile_pool(name="popool", bufs=2, space="PSUM") as popool,
        ):
            # ---- persistent SBUF tiles ----
            xt_sb = cpool.tile([128, DT, S], bf16, name="xt_sb")
            wq_sb = cpool.tile([128, DT, HG_D], bf16, name="wq_sb")
            wk_sb = cpool.tile([128, DT, HG_D], bf16, name="wk_sb")
            wv_sb = cpool.tile([128, DT, HG_D], bf16, name="wv_sb")
            wo_sb = cpool.tile([128, OT, D], bf16, name="wo_sb")
            bq_sb = cpool.tile([128, OT], f32, name="bq_sb")
            bk_sb = cpool.tile([128, OT], f32, name="bk_sb")
            bvr_sb = cpool.tile([1, HG_D], bf16, name="bvr_sb")
            bor_sb = cpool.tile([1, D], bf16, name="bor_sb")
            ones_t = cpool.tile([128, 128], bf16, name="ones_t")
            qT_sb = cpool.tile([128, OT, S], bf16, name="qT_sb")
            kT_sb = cpool.tile([128, OT, S], bf16, name="kT_sb")
            # v with a trailing ones column per head: attnv lhsT [128, 65]
            # whose 65th output row accumulates the softmax denominator.
            v_sb = cpool.tile([128, ST, NH, 65], bf16, name="v_sb")
            aoT_sb = cpool.tile([128, OT, S], bf16, name="aoT_sb")

            # ---- loads ----
            for k in range(DT):
                nc.sync.dma_start(out=xt_sb[:, k, :], in_=xt_d[k * 128:(k + 1) * 128, :])
                nc.sync.dma_start(out=wq_sb[:, k, :], in_=wq_d[k * 128:(k + 1) * 128, :])
                nc.sync.dma_start(out=wk_sb[:, k, :], in_=wk_d[k * 128:(k + 1) * 128, :])
                nc.sync.dma_start(out=wv_sb[:, k, :], in_=wv_d[k * 128:(k + 1) * 128, :])
            for k in range(OT):
                nc.sync.dma_start(out=wo_sb[:, k, :], in_=wo_d[k * 128:(k + 1) * 128, :])
            nc.sync.dma_start(out=bq_sb[:], in_=bq_d[:])
            nc.sync.dma_start(out=bk_sb[:], in_=bk_d[:])
            nc.sync.dma_start(out=bvr_sb[:], in_=bv_d[:])
            nc.sync.dma_start(out=bor_sb[:], in_=bo_d[:])
            nc.gpsimd.memset(ones_t[:], 1.0)

            # ---- projections: qT/kT [d-major], bias via per-partition scalar ----
            for w_sb, b_sb, out_sb in ((wq_sb, bq_sb, qT_sb), (wk_sb, bk_sb, kT_sb)):
                for p in range(OT):
                    for jc in range(4):
                        pq = pspool.tile([128, 512], f32, tag="ps", name="pq")
                        for k in range(DT):
                            nc.tensor.matmul(
                                pq[:],
                                w_sb[:, k, p * 128:(p + 1) * 128],
                                xt_sb[:, k, jc * 512:(jc + 1) * 512],
                                start=(k == 0), stop=(k == DT - 1),
                            )
                        nc.vector.tensor_scalar_add(
                            out_sb[:, p, jc * 512:(jc + 1) * 512], pq[:], b_sb[:, p:p + 1])

            # ---- v projection [s-major], bias via K=1 ones matmul ----
            nc.vector.memset(v_sb[:], 1.0)
            for st in range(ST):
                pv = pspool.tile([128, 512], f32, tag="ps", name="pv")
                for k in range(DT):
                    nc.tensor.matmul(
                        pv[:],
                        xt_sb[:, k, st * 128:(st + 1) * 128],
                        wv_sb[:, k, :],
                        start=(k == 0), stop=False,
                    )
                nc.tensor.matmul(pv[:], ones_t[0:1, 0:128], bvr_sb[0:1, :],
                                 start=False, stop=True)
                nc.vector.tensor_copy(
                    v_sb[:, st, :, 0:64],
                    pv.rearrange("p (h c) -> p h c", c=64))

            # ---- attention, head by head ----
            # Normalization (broadcast of 1/denom + scale) is deferred by one
            # (h, j) chunk so the PE never waits on the slow DVE reciprocal.
            pending = []

            def flush_normalize():
                off2, p2, jb2, rb2 = pending.pop(0)
                bt = pspool.tile([128, 1024], f32, tag="ps", name="bt")
                for l in range(2):
                    nc.tensor.matmul(
                        bt[off2:off2 + 64, l * 512:(l + 1) * 512],
                        ones_t[64:65, 0:64],
                        rb2[64:65, l * 512:(l + 1) * 512],
                        start=True, stop=True,
                    )
                nc.vector.tensor_mul(
                    aoT_sb[off2:off2 + 64, p2, jb2],
                    aoT_sb[off2:off2 + 64, p2, jb2],
                    bt[off2:off2 + 64, :])

            for h in range(NH):
                p, hh = h // 2, h % 2
                off = 64 * hh
                for j in range(2):
                    jb = slice(j * 1024, (j + 1) * 1024)
                    ot = popool.tile([65, 1024], f32, tag="po", name="ot")
                    for i in range(KT):
                        stt = pspool.tile([128, 1024], f32, tag="ps", name="stt")
                        for l in range(2):
                            nc.tensor.matmul(
                                stt[:, l * 512:(l + 1) * 512],
                                kT_sb[off:off + 64, p, i * 128:(i + 1) * 128],
                                qT_sb[off:off + 64, p,
                                      j * 1024 + l * 512:j * 1024 + (l + 1) * 512],
                                start=True, stop=True,
                            )
                        et = wpool.tile([128, 1024], bf16, tag="exp", bufs=3,
                                        name="et")
                        nc.scalar.activation(et[:], stt[:], Exp, scale=0.125)
                        for l in range(2):
                            nc.tensor.matmul(
                                ot[:, l * 512:(l + 1) * 512],
                                v_sb[:, i, h, :],
                                et[:, l * 512:(l + 1) * 512],
                                start=(i == 0), stop=(i == KT - 1),
                            )
                    # drain attn rows (cross-partition for odd heads) and
                    # compute 1/denom from row 64.
                    nc.vector.tensor_copy(aoT_sb[off:off + 64, p, jb], ot[0:64, :])
                    rf = wpool.tile([65, 1024], f32, tag="rf", name="rf")
                    rb = wpool.tile([65, 1024], bf16, tag="rb", bufs=3, name="rb")
                    nc.vector.reciprocal(rf[64:65, :], ot[64:65, :])
                    nc.vector.tensor_copy(rb[64:65, :], rf[64:65, :])
                    pending.append((off, p, jb, rb))
                    if len(pending) > 1:
                        flush_normalize()
            while pending:
                flush_normalize()

            # ---- output projection + bias ----
            for st in range(ST):
                yt = wpool.tile([128, D], f32, tag="y", name="yt")
                for l in range(2):
                    py = pspool.tile([128, 512], f32, tag="ps", name="py")
                    for kt in range(OT):
                        nc.tensor.matmul(
                            py[:],
                            aoT_sb[:, kt, st * 128:(st + 1) * 128],
                            wo_sb[:, kt, l * 512:(l + 1) * 512],
                            start=(kt == 0), stop=False,
                        )
                    nc.tensor.matmul(py[:], ones_t[0:1, 0:128],
                                     bor_sb[0:1, l * 512:(l + 1) * 512],
                                     start=False, stop=True)
                    nc.vector.tensor_copy(yt[:, l * 512:(l + 1) * 512], py[:])
                nc.sync.dma_start(out=y_d[st * 128:(st + 1) * 128, :], in_=yt[:])

    nc.compile()
    return nc


def get_nc():
    global _CACHED_NC
    if _CACHED_NC is None:
        _CACHED_NC = _build_nc()
    return _CACHED_NC


def make_in_maps(x, Wq, bq, Wk, bk, Wv, bv, Wo, bo):
    x = np.asarray(x, dtype=np.float32)
    in_maps = []
    for c in range(N_CORES):
        b, hg = c // 2, c % 2
        sl = slice(hg * HG_D, (hg + 1) * HG_D)
        in_maps.append({
            "xt": np.ascontiguousarray(np.asarray(x[b]).T).astype(BF16),
            "wq": np.ascontiguousarray(np.asarray(Wq)[:, sl]).astype(BF16),
            "wk": np.ascontiguousarray(np.asarray(Wk)[:, sl]).astype(BF16),
            "wv": np.ascontiguousarray(np.asarray(Wv)[:, sl]).astype(BF16),
            "wo": np.ascontiguousarray(np.asarray(Wo)[sl, :]).astype(BF16),
            "bqt": np.ascontiguousarray(
                np.asarray(bq, np.float32)[sl].reshape(OT, 128).T),
            "bkt": np.ascontiguousarray(
                np.asarray(bk, np.float32)[sl].reshape(OT, 128).T),
            "bvr": np.asarray(bv, np.float32)[sl].reshape(1, HG_D).astype(BF16),
            "bor": (np.asarray(bo, np.float32) if hg == 0
                    else np.zeros(D, np.float32)).reshape(1, D).astype(BF16),
        })
    return in_maps


def run_cores(in_maps, trace=False):
    try:
        import ntff_shim
        ntff_shim.install()
    except Exception:
        pass
    from concourse.bass_utils import run_bass_kernel_spmd

    nc = get_nc()
    return run_bass_kernel_spmd(nc, in_maps, list(range(N_CORES)), trace=trace)


def combine(results):
    y = np.empty((4, S, D), np.float32)
    for b in range(4):
        y[b] = results[2 * b]["y"] + results[2 * b + 1]["y"]
    return y


def kernel(x, Wq, bq, Wk, bk, Wv, bv, Wo, bo):
    in_maps = make_in_maps(x, Wq, bq, Wk, bk, Wv, bv, Wo, bo)
    res = run_cores(in_maps, trace=False)
    return combine(res.results)
